# revision 27
# baseline (speedup 1.0000x reference)
# Trainium2 Bass/Tile kernel for the sparse cosine-similarity multi-head
# attention module (B=16, NT=NC=E=512, H=8, DK=DV=64).
#
# Sharding: data-parallel over batch — each of the 8 cores processes 2 batches
# end-to-end (no collectives). Host transposes inputs once so every matmul
# contraction lands on the partition dimension.
#
# Per-core dataflow (all fp32):
#   qT[hd,t] = At_w @ queries[b].T   (+bias via K=1 rank-1 matmul into PSUM)
#   kT[hd,c], v[c,hd] likewise
#   qn2[h,t]/kn2[h,c] = blockwise sum of squares (indicator matmul on squared
#     projections); 1/qn, 1/kn via exp(-0.5*ln(.)) on ScalarE
#   q is pre-scaled by 1/qn (PE-broadcast of ln, exp, then DVE multiply)
#   dots[c,t] = (k_h)^T q_h ; E = exp(dots*scale[c] + bias[c]) where
#     scale[c]=1/kn[h,c], bias[c]=pos_bias[c] or -1e30 at masked c (exp -> 0)
#   U[65,t] = [v_h | 1]^T @ E  -> row 64 is the softmax denominator s[t]
#   rb[*,t] = exp(-ln s[t]) broadcast to 128 partitions via K=1 matmul
#   att[c,t] = E * rb (written transposed; host returns a transposed view)
#   OT[hd,t] = U[:64] * rb ; out[t,e] = OT^T @ R_w^T + R_b (bias via K=1 mm)
import numpy as np

B, NT, NC, E, H, DK, DV = 16, 512, 512, 512, 8, 64, 64
N_CORES = 8
BPC = B // N_CORES  # batches per core
NEG = -1.0e30

_cache = {}


def _build():
    import concourse.mybir as mybir
    import concourse.tile as tile
    from concourse import bacc

    f32 = mybir.dt.float32
    nc = bacc.Bacc(
        "TRN2",
        target_bir_lowering=False,
        debug=False,
        enable_asserts=False,
        num_devices=N_CORES,
    )

    # DRAM I/O (per-core shapes)
    qTd_h = nc.dram_tensor("qTd", [BPC, E, NT], f32, kind="ExternalInput").ap()
    kTd_h = nc.dram_tensor("kTd", [BPC, E, NC], f32, kind="ExternalInput").ap()
    vTd_h = nc.dram_tensor("vTd", [BPC, E, NC], f32, kind="ExternalInput").ap()
    wq_h = nc.dram_tensor("wq", [E, H * DK], f32, kind="ExternalInput").ap()
    wk_h = nc.dram_tensor("wk", [E, H * DK], f32, kind="ExternalInput").ap()
    wv_h = nc.dram_tensor("wv", [E, H * DV], f32, kind="ExternalInput").ap()
    rwt_h = nc.dram_tensor("rwt", [H * DV, E], f32, kind="ExternalInput").ap()
    biases_h = nc.dram_tensor("biases", [1, 4 * E], f32, kind="ExternalInput").ap()
    # constant masks, host-built: cols = selT(32) | sel2(2) | ident(128)
    cst128_h = nc.dram_tensor("cst128", [128, 162], f32, kind="ExternalInput").ap()
    # row constants: row0[0:512] = ones; rows 0:2 cols 512:640 = sel2T
    cst2_h = nc.dram_tensor("cst2", [2, 640], f32, kind="ExternalInput").ap()
    pbm_h = nc.dram_tensor("pbm", [BPC, 128, 4], f32, kind="ExternalInput").ap()
    out_h = nc.dram_tensor("out", [BPC, NT, E], f32, kind="ExternalOutput").ap()
    attT_h = nc.dram_tensor("attT", [BPC, H, NC, NT], f32, kind="ExternalOutput").ap()

    KT = E // 128  # contraction tiles (4)
    MT = 4         # output row tiles of 128

    with tile.TileContext(nc) as tc:
        with (
            tc.tile_pool(name="consts", bufs=1) as consts,
            tc.tile_pool(name="inp", bufs=1) as inp,
            tc.tile_pool(name="proj", bufs=2) as proj,
            tc.tile_pool(name="sq", bufs=1) as sqp,
            tc.tile_pool(name="smallv", bufs=4) as smallv,
            tc.tile_pool(name="lnsp", bufs=2) as lnsp,
            tc.tile_pool(name="qmul", bufs=2) as qmulp,
            tc.tile_pool(name="epool", bufs=2) as epool,
            tc.tile_pool(name="rbp", bufs=2) as rbp,
            tc.tile_pool(name="attst", bufs=4) as attst,
            tc.tile_pool(name="pp", bufs=4, space="PSUM") as pp,
            tc.tile_pool(name="up", bufs=2, space="PSUM") as up,
            tc.tile_pool(name="sp", bufs=2, space="PSUM") as sp,
        ):
            # ---- constants ----
            wq = consts.tile([128, KT * 512], f32, tag="wq")
            wk = consts.tile([128, KT * 512], f32, tag="wk")
            wv = consts.tile([128, KT * 512], f32, tag="wv")
            rwt = consts.tile([128, KT * 512], f32, tag="rwt")
            def load_kt(dst, src):
                # [512, 512] HBM -> [128, 4*512] SBUF with kt-block columns
                nc.sync.dma_start(
                    dst.rearrange("p (kt t) -> p kt t", kt=KT),
                    src.rearrange("(kt p) t -> p kt t", p=128))

            load_kt(wq[:, :], wq_h[:, :])
            load_kt(wk[:, :], wk_h[:, :])
            load_kt(wv[:, :], wv_h[:, :])
            load_kt(rwt[:, :], rwt_h[:, :])
            # biases on partition 0: blocks of 512 = At_b, Ac_b, Bc_b, R_b
            biases = consts.tile([1, 4 * E], f32, tag="biases")
            nc.sync.dma_start(biases[:, :], biases_h[:, :])
            pbm = consts.tile([128, BPC * 4], f32, tag="pbm")
            for b in range(BPC):
                nc.sync.dma_start(pbm[:, b * 4:(b + 1) * 4], pbm_h[b, :, :])
            cst128 = consts.tile([128, 162], f32, tag="cst128")
            nc.sync.dma_start(cst128[:, :], cst128_h[:, :])
            cst2 = consts.tile([2, 640], f32, tag="cst2")
            nc.sync.dma_start(cst2[:, :], cst2_h[:, :])
            selT = cst128[:, 0:32]    # selT[p, mt*8+h] = (h == 2mt + p//64)
            sel2 = cst128[:, 32:34]   # sel2[p, j] = (j == p//64)
            ident = cst128[:, 34:162]
            ones = cst2[0:1, 0:512]
            sel2T = cst2[0:2, 512:640]

            for b in range(BPC):
                # ---- load inputs (transposed on host: [E, N]) ----
                qTd = inp.tile([128, KT * 512], f32, tag="qTd")
                kTd = inp.tile([128, KT * 512], f32, tag="kTd")
                vTd = inp.tile([128, KT * 512], f32, tag="vTd")
                load_kt(qTd[:, :], qTd_h[b])
                load_kt(kTd[:, :], kTd_h[b])
                load_kt(vTd[:, :], vTd_h[b])

                # ---- projections + norms ----
                qTs = proj.tile([128, MT * 512], f32, tag="qTs")   # scaled qT [hd, t]
                kTt = proj.tile([128, MT * 512], f32, tag="kTt")   # kT [hd, c]
                vaug = proj.tile([128, MT * 520], f32, tag="vaug")  # per c-tile: 8*(64 v + 1 ones)
                qsq = sqp.tile([128, MT * 512], f32, tag="qsq")
                ksq = sqp.tile([128, MT * 512], f32, tag="ksq")
                kn2 = sp.tile([128, 512], f32, tag="sp")
                lnk = smallv.tile([8, 512], f32, tag="smallv")
                knr = smallv.tile([8, 512], f32, tag="smallv")

                for mt in range(MT):
                    # q projection: out[hd-tile, t]
                    q_ps = pp.tile([128, 512], f32, tag="pp")
                    for kt in range(KT):
                        nc.tensor.matmul(
                            q_ps[:, :], wq[:, kt * 512 + mt * 128: kt * 512 + (mt + 1) * 128],
                            qTd[:, kt * 512:(kt + 1) * 512], start=(kt == 0), stop=False)
                    nc.tensor.matmul(q_ps[:, :], biases[0:1, mt * 128:(mt + 1) * 128],
                                     ones[0:1, :], start=False, stop=True,
                                     skip_group_check=True)
                    # k projection
                    k_ps = pp.tile([128, 512], f32, tag="pp")
                    for kt in range(KT):
                        nc.tensor.matmul(
                            k_ps[:, :], wk[:, kt * 512 + mt * 128: kt * 512 + (mt + 1) * 128],
                            kTd[:, kt * 512:(kt + 1) * 512], start=(kt == 0), stop=False)
                    nc.tensor.matmul(k_ps[:, :], biases[0:1, 512 + mt * 128: 512 + (mt + 1) * 128],
                                     ones[0:1, :], start=False, stop=True,
                                     skip_group_check=True)
                    # v projection: out[c-tile, hd]
                    v_ps = pp.tile([128, 512], f32, tag="pp")
                    for kt in range(KT):
                        nc.tensor.matmul(
                            v_ps[:, :], vTd[:, kt * 512 + mt * 128: kt * 512 + (mt + 1) * 128],
                            wv[:, kt * 512:(kt + 1) * 512], start=(kt == 0), stop=False)
                    nc.tensor.matmul(v_ps[:, :], ones[0:1, 0:128],
                                     biases[0:1, 1024:1536], start=False, stop=True,
                                     skip_group_check=True)

                    # squares (ScalarE) for norms
                    nc.scalar.square(qsq[:, mt * 512:(mt + 1) * 512], q_ps[:, :])
                    nc.scalar.square(ksq[:, mt * 512:(mt + 1) * 512], k_ps[:, :])
                    # q-norm pipeline, fully per-mt: qn2 [2,512] -> ln -> bcast -> exp -> scale
                    qn2m = sp.tile([128, 512], f32, tag="sp")
                    nc.tensor.matmul(qn2m[0:2, :], sel2[:, :],
                                     qsq[:, mt * 512:(mt + 1) * 512], start=True, stop=True)
                    lnqm = smallv.tile([8, 512], f32, tag="smallv")
                    nc.scalar.activation(lnqm[0:2, :], qn2m[0:2, :],
                                         mybir.ActivationFunctionType.Ln)
                    ex_ps = pp.tile([128, 512], f32, tag="pp")
                    nc.tensor.matmul(ex_ps[:, :], sel2T[:, :], lnqm[0:2, :],
                                     start=True, stop=True)
                    qmul = qmulp.tile([128, 512], f32, tag="qmul")
                    nc.scalar.activation(qmul[:, :], ex_ps[:, :],
                                         mybir.ActivationFunctionType.Exp, scale=-0.5)
                    nc.vector.tensor_mul(qTs[:, mt * 512:(mt + 1) * 512],
                                         q_ps[:, :], qmul[:, :])
                    # k-norm reduction accumulates across mt
                    nc.tensor.matmul(kn2[0:8, :], selT[:, mt * 8:(mt + 1) * 8],
                                     ksq[:, mt * 512:(mt + 1) * 512],
                                     start=(mt == 0), stop=(mt == MT - 1))
                    # k copy to SBUF (DVE); v copy with head-interleaved ones column
                    nc.vector.tensor_copy(kTt[:, mt * 512:(mt + 1) * 512], k_ps[:, :])
                    vdst = vaug[:, mt * 520:(mt + 1) * 520].rearrange("p (h x) -> p h x", h=H)
                    nc.vector.tensor_copy(
                        vdst[:, :, 0:64],
                        v_ps[:, :].rearrange("p (h x) -> p h x", h=H))
                nc.vector.memset(
                    vaug[:, :].rearrange("p (m h x) -> p m h x", m=MT, h=H)[:, :, :, 64:65], 1.0)

                # 1/kn as per-partition columns
                nc.scalar.activation(lnk[:, :], kn2[0:8, :], mybir.ActivationFunctionType.Ln)
                nc.scalar.activation(knr[:, :], lnk[:, :], mybir.ActivationFunctionType.Exp,
                                     scale=-0.5)
                kncol = smallv.tile([128, 32], f32, tag="kncol")  # [c-part, ct*8 + h]
                for ct in range(4):
                    tp = sp.tile([128, 512], f32, tag="sp")
                    nc.tensor.transpose(tp[0:128, 0:8], knr[:, ct * 128:(ct + 1) * 128],
                                        ident[0:8, 0:8])
                    nc.scalar.copy(kncol[:, ct * 8:(ct + 1) * 8], tp[0:128, 0:8])

                # ---- attention per head ----
                OT = proj.tile([128, MT * 512], f32, tag="OT")  # [hd, t]
                for h in range(H):
                    po = (h % 2) * 64       # partition offset of head h
                    bo = (h // 2) * 512     # free-dim block of head h
                    e_t = epool.tile([128, 4 * 512], f32, tag="E")
                    u_ps = up.tile([128, 512], f32, tag="up")
                    for ct in range(4):
                        d_ps = pp.tile([128, 512], f32, tag="pp")
                        nc.tensor.matmul(
                            d_ps[:, :],
                            kTt[po:po + 64, bo + ct * 128: bo + (ct + 1) * 128],
                            qTs[po:po + 64, bo:bo + 512], start=True, stop=True)
                        nc.scalar.activation(
                            e_t[:, ct * 512:(ct + 1) * 512], d_ps[:, :],
                            mybir.ActivationFunctionType.Exp,
                            bias=pbm[:, b * 4 + ct: b * 4 + ct + 1],
                            scale=kncol[:, ct * 8 + h: ct * 8 + h + 1])
                        nc.tensor.matmul(
                            u_ps[0:65, :],
                            vaug[:, ct * 520 + h * 65: ct * 520 + (h + 1) * 65],
                            e_t[:, ct * 512:(ct + 1) * 512],
                            start=(ct == 0), stop=(ct == 3))
                    lnS = lnsp.tile([1, 512], f32, tag="lnS")
                    nc.scalar.activation(lnS[:, :], u_ps[64:65, :],
                                         mybir.ActivationFunctionType.Ln)
                    rb_ps = sp.tile([128, 512], f32, tag="sp")
                    nc.tensor.matmul(rb_ps[:, :], ones[0:1, 0:128], lnS[:, :],
                                     start=True, stop=True)
                    rb = rbp.tile([128, 512], f32, tag="rb")
                    nc.scalar.activation(rb[:, :], rb_ps[:, :],
                                         mybir.ActivationFunctionType.Exp, scale=-1.0)
                    nc.vector.tensor_mul(OT[po:po + 64, bo:bo + 512],
                                         u_ps[0:64, :], rb[0:64, :])
                    for ct in range(4):
                        att_sb = attst.tile([128, 512], f32, tag="att")
                        nc.vector.tensor_mul(att_sb[:, :],
                                             e_t[:, ct * 512:(ct + 1) * 512], rb[:, :])
                        nc.sync.dma_start(
                            attT_h[b, h, ct * 128:(ct + 1) * 128, :], att_sb[:, :])

                # ---- output projection ----
                for tt in range(MT):
                    r_ps = pp.tile([128, 512], f32, tag="pp")
                    for kt in range(KT):
                        nc.tensor.matmul(
                            r_ps[:, :],
                            OT[:, kt * 512 + tt * 128: kt * 512 + (tt + 1) * 128],
                            rwt[:, kt * 512:(kt + 1) * 512], start=(kt == 0), stop=False)
                    nc.tensor.matmul(r_ps[:, :], ones[0:1, 0:128], biases[0:1, 1536:2048],
                                     start=False, stop=True, skip_group_check=True)
                    out_sb = attst.tile([128, 512], f32, tag="att")
                    nc.scalar.copy(out_sb[:, :], r_ps[:, :])
                    nc.sync.dma_start(out_h[b, tt * 128:(tt + 1) * 128, :], out_sb[:, :])
    nc.compile()
    return nc


def _get_nc():
    if "nc" not in _cache:
        _cache["nc"] = _build()
    return _cache["nc"]


def kernel(queries, keys, values, At_w, At_b, Ac_w, Ac_b, Bc_w, Bc_b,
           pos_bias, R_w, R_b, attention_mask):
    from concourse.bass_utils import run_bass_kernel_spmd

    nc = _get_nc()
    f = np.float32
    qT = np.ascontiguousarray(np.asarray(queries, f).transpose(0, 2, 1))
    kT = np.ascontiguousarray(np.asarray(keys, f).transpose(0, 2, 1))
    vT = np.ascontiguousarray(np.asarray(values, f).transpose(0, 2, 1))
    wq = np.ascontiguousarray(np.asarray(At_w, f).T)
    wk = np.ascontiguousarray(np.asarray(Ac_w, f).T)
    wv = np.ascontiguousarray(np.asarray(Bc_w, f).T)
    rwt = np.ascontiguousarray(np.asarray(R_w, f).T)
    biases = np.concatenate(
        [np.asarray(At_b, f), np.asarray(Ac_b, f),
         np.asarray(Bc_b, f), np.asarray(R_b, f)]).reshape(1, 4 * E)
    pbm_full = np.where(np.asarray(attention_mask, bool), np.float32(NEG),
                        np.asarray(pos_bias, f)[None, :]).astype(f)  # [B, NC]
    # [B, NC] -> [B, 128, 4] so that column ct holds c = ct*128 + p
    pbm_cols = np.ascontiguousarray(pbm_full.reshape(B, 4, 128).transpose(0, 2, 1))

    # constant masks
    p_idx = np.arange(128)
    selT = np.zeros((128, 32), f)  # selT[p, mt*8+h] = (h == 2mt + p//64)
    for mt in range(4):
        selT[p_idx, mt * 8 + 2 * mt + p_idx // 64] = 1.0
    sel2 = np.zeros((128, 2), f)
    sel2[p_idx, p_idx // 64] = 1.0
    cst128 = np.concatenate([selT, sel2, np.eye(128, dtype=f)], axis=1)
    cst2 = np.zeros((2, 640), f)
    cst2[0, 0:512] = 1.0
    cst2[0:2, 512:640] = sel2.T

    in_maps = []
    for i in range(N_CORES):
        s = slice(i * BPC, (i + 1) * BPC)
        in_maps.append({
            "qTd": qT[s], "kTd": kT[s], "vTd": vT[s],
            "wq": wq, "wk": wk, "wv": wv, "rwt": rwt,
            "biases": biases, "pbm": pbm_cols[s],
            "cst128": cst128, "cst2": cst2,
        })
    res = run_bass_kernel_spmd(nc, in_maps, list(range(N_CORES)))
    out = np.concatenate([r["out"] for r in res.results], axis=0)
    attT = np.concatenate([r["attT"] for r in res.results], axis=0)
    return out, attT.transpose(0, 1, 3, 2)
